# revision 1
# baseline (speedup 1.0000x reference)
"""ChebyKAN linear layer on 8 Trainium2 NeuronCores.

Math: y[b,o] = sum_{i,d} T_d(w[b,i]) * C[i,o,d], with w = tanh(tanh(x)) and
T_d the Chebyshev polynomials. The device evaluates the Chebyshev-product
basis phi = [T1, T1^2, T1*T2, T2^2, T2*T3, T3^2, T3*T4, T4^2]; an exact
host-side linear transform maps Chebyshev coefficients onto this basis
(constant column folds into a per-o bias added during PSUM evacuation),
with the basis axis permuted into the device consumption order J_ORDER.

Sharding: data-parallel over batch b (16384 -> 2048/core); coeffs
replicated. x is pre-laid-out on the host as [128 part, phase, ib, 512].

Matmul operands are bf16 (1 cycle/row like f32r at free-dim 512, but half
the DMA + SBUF read traffic and 2x faster FWL weight loads). The basis
chain stays fp32 on ACT/DVE; each matmul operand is rounded to bf16
exactly once. Output is stored bf16 and widened on the host. Measured
rel err 4.5e-3 (gate 2e-2); PE stream runs at 216 ns per 512-row matmul
vs the 213.3 ns roofline.

Engine layout per (phase, ib) block:
  ACT: tanh, tanh, Sq(t1)=f2, Sq(2f2-1)=f4, Sq(2f4-1)=f8, Sq(t3)=f6
       (the two affines fold into the Square pre-scale/bias)
  DVE: casts t1b/f2b/f4b, TS t2/u3/t4, TT t3/f3/f5/f7, and a 1-column
       zero "guard" per block (see below)
  PE:  8 basis cols x 4 output blocks, j-major; last row-block oc-major
       so the four PSUM groups finish staggered and evacuation overlaps
       the stream; the final group runs in column halves so the very
       last store moves only 64KB

Hard-won scheduling rules encoded here (the Tile scheduler's DMA model is
optimistic, engine queues are in-order, and HAM re-throttles the core ~2us
after any PE idle):
  - every DMA-gated ACT op (the in-place tanh of each block) takes a bias
    operand derived from the previous block's last ACT output, so the
    scheduler cannot hoist it ahead of the running chain and head-of-line
    block the ACT queue on a late transfer
  - x phases 1-3 ride the Sync ring BEHIND all W: anything sizable on the
    Pool SWDGE ring early starves the shared DMA-engine pool exactly when
    the critical W stream needs it
  - warm-up dummy matmuls cover the PE from engine-ready (~7.4us) to the
    first real matmul (~13us) so the clock gate never drops; a few more
    dummies gated on the first half-evacuation hold full clock through the
    tail without extending the PE drain past the final store

Residual (measured, near-irreducible here): ~8.3us runtime preamble before
the first DMA packet, ~3.3us first-sliver transfer + completion semaphore,
~1.6us serial tanh/square chain ramp (the post-gap matmuls all wait on the
DVE basis semaphore, not weights), 2.7ns/matmul over the PE roofline, and
~3.4us TileContext teardown.
"""

import sys

if "/opt/trn_rl_repo" not in sys.path:
    sys.path.append("/opt/trn_rl_repo")

import ml_dtypes
import numpy as np

import concourse.bacc as bacc
import concourse.tile as tile
from concourse import mybir
from concourse.bass_utils import run_bass_kernel_spmd

DEGREE = 8
B, C_IN, C_OUT = 16384, 512, 512
N_CORES = 8
NB = B // N_CORES            # 2048 batch rows per core
B_TILE = 512                 # batch window per PSUM accumulation phase
N_PHASES = NB // B_TILE      # 4
N_IB = C_IN // 128           # 4 contraction row-blocks
N_J = DEGREE                 # basis funcs phi_1..phi_8 (constant -> bias)
F32 = mybir.dt.float32
F16 = mybir.dt.float16
BF16 = mybir.dt.bfloat16

_CACHE = {}

# per-ib matmul consumption order of the basis functions, by readiness:
# t1b, f2b first, then the fused-ACT f4/f8 and the DVE products
J_ORDER = [0, 1, 3, 2, 7, 4, 5, 6]


def _build():
    nc = bacc.Bacc("TRN2", target_bir_lowering=False, debug=False)
    xh = nc.dram_tensor("xh", [128, N_PHASES, N_IB, B_TILE], F32, kind="ExternalInput")
    wmat = nc.dram_tensor("wmat", [C_IN, N_J * C_OUT], BF16, kind="ExternalInput")
    biasv = nc.dram_tensor("biasv", [128, 4], F32, kind="ExternalInput")
    yt = nc.dram_tensor("yt", [C_OUT, NB], BF16, kind="ExternalOutput")

    Tanh = mybir.ActivationFunctionType.Tanh
    Square = mybir.ActivationFunctionType.Square
    Identity = mybir.ActivationFunctionType.Identity
    ALU_MULT = mybir.AluOpType.mult
    ALU_ADD = mybir.AluOpType.add

    with tile.TileContext(nc) as tc:
        with (
            tc.tile_pool(name="const", bufs=1) as const_pool,
            tc.tile_pool(name="wts", bufs=1) as wpool,
            tc.tile_pool(name="pows", bufs=2) as ppool,
            tc.tile_pool(name="outs", bufs=2) as opool,
            tc.tile_pool(name="psum", bufs=2, space="PSUM") as pspool,
        ):
            # PE warm-up fodder: dummy matmuls on a memset tile hold the
            # HAM clock gate at full speed until the real stream is ready.
            dummy = const_pool.tile([128, B_TILE], BF16, tag="dummy")
            nc.gpsimd.memset(dummy[:], 0.0)
            dps = pspool.tile([128, B_TILE], F32, tag="ps3", name="dps")
            for _ in range(15):
                nc.tensor.matmul(
                    dps[:], lhsT=dummy[:, 0:128], rhs=dummy[:],
                    start=True, stop=True,
                )

            # DMA rings (first packets ~8.3us, after the fixed preamble):
            #   Sync ring (HWDGE):   bias (engine warm-up), x phase-0
            #                        slivers interleaved with W pairs in PE
            #                        consumption order, then x phases 1-3,
            #                        then the last-phase stores
            #   Scalar ring (HWDGE): W pairs whose need-time has slack
            #   Pool ring (SWDGE):   phase 0-2 stores only (anything early
            #                        here starves the shared engine pool)
            w_sb = {}

            def w_src(ib):
                return wmat.ap()[ib * 128 : (ib + 1) * 128, :].rearrange(
                    "p (j o) -> p j o", j=N_J
                )

            def load_w(ib, m, eng):
                wc = wpool.tile(
                    [128, 2, C_OUT], BF16, tag=f"w{ib}_{m}", name=f"w{ib}_{m}"
                )
                eng.dma_start(out=wc[:], in_=w_src(ib)[:, 2 * m : 2 * m + 2, :])
                w_sb[ib, 2 * m] = (wc, 0)
                w_sb[ib, 2 * m + 1] = (wc, 1)

            xlbs = []
            xlb0 = ppool.tile([128, N_IB, B_TILE], F32, tag="xlb0", bufs=1)

            def load_sliver(ib, eng):
                eng.dma_start(out=xlb0[:, ib, :], in_=xh.ap()[:, 0, ib, :])

            # Sync (fast) ring carries the critical-path stream: sliver 0,
            # then W pairs interleaved with the remaining slivers in PE
            # consumption order. Scalar (slower) ring takes chunks whose
            # need-time is comfortably later.
            bias_t = const_pool.tile([128, 4], F32)
            nc.gpsimd.dma_start(out=bias_t[:], in_=biasv.ap())
            load_sliver(0, nc.sync)
            load_w(0, 0, nc.sync)
            load_w(0, 2, nc.scalar)
            load_w(0, 1, nc.sync)
            load_w(0, 3, nc.sync)
            load_sliver(1, nc.sync)
            load_w(1, 1, nc.scalar)
            load_w(1, 0, nc.sync)
            load_sliver(2, nc.sync)
            load_w(1, 3, nc.scalar)
            load_w(1, 2, nc.sync)
            load_sliver(3, nc.sync)
            for ib in range(2, N_IB):
                for m in range(N_J // 2):
                    # odd row-block-3 pairs ride the scalar ring to thin
                    # out the sync queue's mid-window
                    eng = nc.scalar if (ib == 3 and m % 2 == 1) else nc.sync
                    load_w(ib, m, eng)
            xlbs.append(xlb0)

            # x phases 1-3 ride the Sync ring BEHIND all W: they are not
            # needed until ~39/66/93us, and putting them on the Pool SWDGE
            # ring early starves the shared DMA-engine pool right when the
            # critical W stream needs it.
            for ph in range(1, N_PHASES):
                xlb = ppool.tile(
                    [128, N_IB, B_TILE], F32, tag=f"xlb{ph}", bufs=1,
                    name=f"xlb{ph}",
                )
                nc.sync.dma_start(out=xlb[:], in_=xh.ap()[:, ph])
                xlbs.append(xlb)

            def w_chunk(ib, j, oc):
                wc, slot = w_sb[ib, j]
                return wc[:, slot, oc * 128 : (oc + 1) * 128]

            cm1 = const_pool.tile([128, 1], F32, tag="cm1")
            nc.vector.memset(cm1[:], -1.0)

            # ordering guard: each block's first tanh takes a zero-column
            # bias derived from the previous block's LAST ACT output (f6),
            # so the scheduler cannot hoist a DMA-gated tanh ahead of the
            # running chain and head-of-line block the in-order ACT queue
            # on a late transfer
            guard_prev = None

            for ph in range(N_PHASES):
                ps = [
                    pspool.tile([128, B_TILE], F32, tag=f"ps{oc}", name=f"ps{oc}_{ph}")
                    for oc in range(4)
                ]
                bsl = slice(ph * B_TILE, (ph + 1) * B_TILE)
                xlb = xlbs[ph]
                for ib in range(N_IB):
                    # fp32 basis chain; ACT ops in program order:
                    # tanh, tanh, f2, f4, f8, f6 (f6 last: it needs DVE t3)
                    if guard_prev is None:
                        nc.scalar.activation(xlb[:, ib, :], xlb[:, ib, :], Tanh)
                    else:
                        zc = ppool.tile([128, 1], F32, tag="zc", bufs=3)
                        nc.gpsimd.tensor_scalar(
                            zc[:], guard_prev[:, 0:1], 0.0, 0.0, ALU_MULT, ALU_ADD
                        )
                        nc.scalar.activation(
                            xlb[:, ib, :], xlb[:, ib, :], Tanh, bias=zc[:]
                        )
                    t1 = ppool.tile([128, B_TILE], F32, tag="t1", bufs=3)
                    nc.scalar.activation(t1[:], xlb[:, ib, :], Tanh)
                    f2 = ppool.tile([128, B_TILE], F32, tag="f2", bufs=3)
                    nc.scalar.activation(f2[:], t1[:], Square)
                    f4 = ppool.tile([128, B_TILE], F32, tag="f4", bufs=3)
                    nc.scalar.activation(f4[:], f2[:], Square, bias=cm1[:], scale=2.0)
                    f8 = ppool.tile([128, B_TILE], BF16, tag="f8", bufs=3)
                    nc.scalar.activation(f8[:], f4[:], Square, bias=cm1[:], scale=2.0)

                    # DVE: casts + the scalar affines + products
                    t1b = ppool.tile([128, B_TILE], BF16, tag="t1b", bufs=3)
                    nc.vector.tensor_copy(t1b[:], t1[:])
                    t2 = ppool.tile([128, B_TILE], F32, tag="t2", bufs=3)
                    nc.vector.tensor_scalar(t2[:], f2[:], 2.0, -1.0, ALU_MULT, ALU_ADD)
                    u3 = ppool.tile([128, B_TILE], F32, tag="u3", bufs=3)
                    nc.vector.tensor_scalar(u3[:], f2[:], 4.0, -3.0, ALU_MULT, ALU_ADD)
                    f2b = ppool.tile([128, B_TILE], BF16, tag="f2b", bufs=3)
                    nc.vector.tensor_copy(f2b[:], f2[:])
                    t3 = ppool.tile([128, B_TILE], F32, tag="t3", bufs=3)
                    nc.vector.tensor_mul(t3[:], t1[:], u3[:])
                    f3 = ppool.tile([128, B_TILE], BF16, tag="f3", bufs=3)
                    nc.vector.tensor_mul(f3[:], t1[:], t2[:])
                    f4b = ppool.tile([128, B_TILE], BF16, tag="f4b", bufs=3)
                    nc.vector.tensor_copy(f4b[:], f4[:])
                    f5 = ppool.tile([128, B_TILE], BF16, tag="f5", bufs=3)
                    nc.vector.tensor_mul(f5[:], t2[:], t3[:])
                    t4 = ppool.tile([128, B_TILE], F32, tag="t4", bufs=3)
                    nc.vector.tensor_scalar(t4[:], f4[:], 2.0, -1.0, ALU_MULT, ALU_ADD)
                    f6 = ppool.tile([128, B_TILE], BF16, tag="f6", bufs=3)
                    nc.scalar.activation(f6[:], t3[:], Square)
                    guard_prev = f6
                    f7 = ppool.tile([128, B_TILE], BF16, tag="f7", bufs=3)
                    nc.vector.tensor_mul(f7[:], t3[:], t4[:])

                    basis = [t1b, f2b, f3, f4b, f5, f6, f7, f8]
                    # device column jj consumes basis function J_ORDER[jj]
                    chunks = [basis[J_ORDER[jj]] for jj in range(N_J)]
                    out_eng = nc.sync if ph == N_PHASES - 1 else nc.gpsimd

                    def evac(oc, csl):
                        osb = opool.tile(
                            [128, B_TILE], BF16, tag=f"osb{oc}", name=f"osb{oc}"
                        )
                        nc.scalar.activation(
                            osb[:, csl], ps[oc][:, csl], Identity,
                            bias=bias_t[:, oc : oc + 1],
                        )
                        out_eng.dma_start(
                            out=yt.ap()[
                                oc * 128 : (oc + 1) * 128,
                                ph * B_TILE + (csl.start or 0) : ph * B_TILE
                                + (csl.stop or B_TILE),
                            ],
                            in_=osb[:, csl],
                        )
                        return osb

                    if ib < N_IB - 1:
                        for jj in range(N_J):
                            for oc in range(4):
                                nc.tensor.matmul(
                                    ps[oc][:],
                                    lhsT=w_chunk(ib, jj, oc),
                                    rhs=chunks[jj][:],
                                    start=(ib == 0 and jj == 0),
                                    stop=False,
                                )
                    else:
                        # oc-major on the last row-block: accumulation groups
                        # finish staggered -> evacuation overlaps matmuls.
                        for oc in range(3):
                            for jj in range(N_J):
                                nc.tensor.matmul(
                                    ps[oc][:],
                                    lhsT=w_chunk(ib, jj, oc),
                                    rhs=chunks[jj][:],
                                    start=False,
                                    stop=(jj == N_J - 1),
                                )
                            evac(oc, slice(0, B_TILE))
                        # final group in column halves so the very last
                        # evacuation + store move only 64KB; evacuations run
                        # after both halves so the PE never waits on an ACT
                        # read of the still-accumulating PSUM bank
                        for half in range(2):
                            csl = slice(half * 256, half * 256 + 256)
                            for jj in range(N_J):
                                nc.tensor.matmul(
                                    ps[3][:, csl],
                                    lhsT=w_chunk(ib, jj, 3),
                                    rhs=chunks[jj][:, csl],
                                    start=False,
                                    stop=(jj == N_J - 1 and half == 1),
                                    skip_group_check=True,
                                )
                        osb_half_a = evac(3, slice(0, 256))
                        evac(3, slice(256, 512))
                        if ph == N_PHASES - 1:
                            # hold the clock gate through the tail: a few
                            # dummy matmuls gated on the final evacuation
                            # (fresh ps0-tag tile = phase-2's long-idle bank)
                            dps2 = pspool.tile(
                                [128, B_TILE], F32, tag="ps0", name="dps2"
                            )
                            for _ in range(5):
                                nc.tensor.matmul(
                                    dps2[:, 0:256], lhsT=dummy[:, 0:128],
                                    rhs=osb_half_a[:, 0:256],
                                    start=True, stop=True,
                                )
    nc.compile()
    return nc


def _host_transform(cheby_coeffs):
    # Map Chebyshev coefficients onto the device phi basis:
    # phi = [T1, T1^2, T1*T2, T2^2, T2*T3, T3^2, T3*T4, T4^2] and a constant.
    # T_{2k} = 2*T_k^2 - 1, T_{m+n} = 2*T_m*T_n - T_{m-n} =>
    #   y = bias + (C1-C3-C5-C7)*T1 + sum_{d=2..8} 2*C_d * phi_{d-1}
    #   bias_o = sum_i (C0 - C2 - C4 - C6 - C8)
    C64 = cheby_coeffs.astype(np.float64)
    bias = (C64[..., 0] - C64[..., 2] - C64[..., 4] - C64[..., 6] - C64[..., 8]).sum(
        axis=0
    )
    W = np.empty((C_IN, C_OUT, N_J), np.float64)
    W[..., 0] = C64[..., 1] - C64[..., 3] - C64[..., 5] - C64[..., 7]
    for d in range(2, DEGREE + 1):
        W[..., d - 1] = 2.0 * C64[..., d]
    # [i, jj*512+o] with the basis axis permuted into device consumption
    # order (J_ORDER); per-partition-contiguous coefficient rows, bf16
    Wp = W[:, :, J_ORDER]
    Wd = np.ascontiguousarray(
        Wp.transpose(0, 2, 1).reshape(C_IN, N_J * C_OUT).astype(ml_dtypes.bfloat16)
    )
    bias_dev = np.ascontiguousarray(bias.reshape(4, 128).T.astype(np.float32))
    return Wd, bias_dev


def _dev_inputs(x, cheby_coeffs):
    Wd, bias_dev = _host_transform(cheby_coeffs)
    in_maps = []
    for c in range(N_CORES):
        xc = x[c * NB : (c + 1) * NB, :]  # [2048, 512]
        # [p, ph, ib, b] with p the SBUF partition (channel i = ib*128+p)
        xhc = np.ascontiguousarray(
            xc.reshape(N_PHASES, B_TILE, N_IB, 128).transpose(3, 0, 2, 1)
        )
        in_maps.append({"xh": xhc, "wmat": Wd, "biasv": bias_dev})
    return in_maps


def kernel(x, cheby_coeffs):
    x = np.asarray(x, dtype=np.float32)
    cheby_coeffs = np.asarray(cheby_coeffs, dtype=np.float32)
    if "nc" not in _CACHE:
        _CACHE["nc"] = _build()
    nc = _CACHE["nc"]

    in_maps = _dev_inputs(x, cheby_coeffs)
    res = run_bass_kernel_spmd(nc, in_maps, core_ids=list(range(N_CORES)))
    y = np.concatenate(
        [res.results[c]["yt"].T.astype(np.float32) for c in range(N_CORES)], axis=0
    )
    return y



# revision 6
# speedup vs baseline: 1.0265x; 1.0265x over previous
"""ChebyKAN linear layer on 8 Trainium2 NeuronCores.

Math: y[b,o] = sum_{i,d} T_d(w[b,i]) * C[i,o,d], with w = tanh(tanh(x)) and
T_d the Chebyshev polynomials. The device evaluates the Chebyshev-product
basis phi = [T1, T1^2, T1*T2, T2^2, T2*T3, T3^2, T3*T4, T4^2]; an exact
host-side linear transform maps Chebyshev coefficients onto this basis
(constant column folds into a per-o bias added during PSUM evacuation),
with the basis axis permuted into the device consumption order J_ORDER.

Sharding: data-parallel over batch b (16384 -> 2048/core); coeffs
replicated. x is pre-laid-out on the host as [128 part, phase, ib, 512].

Matmul operands are bf16 (1 cycle/row like f32r at free-dim 512, but half
the DMA + SBUF read traffic and 2x faster FWL weight loads). The basis
chain stays fp32 on ACT/DVE; each matmul operand is rounded to bf16
exactly once. Output is stored bf16 and widened on the host. Measured
rel err 4.5e-3 (gate 2e-2); PE stream runs at 216 ns per 512-row matmul
vs the 213.3 ns roofline.

Engine layout per (phase, ib) block:
  ACT: tanh, tanh, Sq(t1)=f2, Sq(2f2-1)=f4, Sq(2f4-1)=f8, Sq(t3)=f6
       (the two affines fold into the Square pre-scale/bias)
  DVE: casts t1b/f2b/f4b, TS t2/u3/t4, TT t3/f3/f5/f7, and a 1-column
       zero "guard" per block (see below)
  PE:  8 basis cols x 4 output blocks, j-major; last row-block oc-major
       so the four PSUM groups finish staggered and evacuation overlaps
       the stream; the final group runs in column halves so the very
       last store moves only 64KB

Hard-won scheduling rules encoded here (the Tile scheduler's DMA model is
optimistic, engine queues are in-order, and HAM re-throttles the core ~2us
after any PE idle):
  - every DMA-gated ACT op (the in-place tanh of each block) takes a bias
    operand derived from the previous block's last ACT output, so the
    scheduler cannot hoist it ahead of the running chain and head-of-line
    block the ACT queue on a late transfer
  - x phases 1-3 ride the Sync ring BEHIND all W: anything sizable on the
    Pool SWDGE ring early starves the shared DMA-engine pool exactly when
    the critical W stream needs it
  - warm-up dummy matmuls cover the PE from engine-ready (~7.4us) to the
    first real matmul (~13us) so the clock gate never drops; a few more
    dummies gated on the first half-evacuation hold full clock through the
    tail without extending the PE drain past the final store

Residual (measured, near-irreducible here): ~8.3us runtime preamble before
the first DMA packet, ~3.3us first-sliver transfer + completion semaphore,
~1.6us serial tanh/square chain ramp (the post-gap matmuls all wait on the
DVE basis semaphore, not weights), 2.7ns/matmul over the PE roofline, and
~3.4us TileContext teardown.
"""

import sys

if "/opt/trn_rl_repo" not in sys.path:
    sys.path.append("/opt/trn_rl_repo")

import ml_dtypes
import numpy as np

import concourse.bacc as bacc
import concourse.tile as tile
from concourse import mybir
from concourse.bass_utils import run_bass_kernel_spmd

DEGREE = 8
B, C_IN, C_OUT = 16384, 512, 512
N_CORES = 8
NB = B // N_CORES            # 2048 batch rows per core
B_TILE = 512                 # batch window per PSUM accumulation phase
N_PHASES = NB // B_TILE      # 4
N_IB = C_IN // 128           # 4 contraction row-blocks
N_J = DEGREE                 # basis funcs phi_1..phi_8 (constant -> bias)
F32 = mybir.dt.float32
F16 = mybir.dt.float16
BF16 = mybir.dt.bfloat16

_CACHE = {}

# per-ib matmul consumption order of the basis functions, by readiness:
# t1b, f2b first, then the fused-ACT f4/f8 and the DVE products
J_ORDER = [0, 1, 3, 2, 7, 4, 5, 6]


def _build():
    nc = bacc.Bacc("TRN2", target_bir_lowering=False, debug=False)
    xh = nc.dram_tensor("xh", [128, N_PHASES, N_IB, B_TILE], F32, kind="ExternalInput")
    wmat = nc.dram_tensor("wmat", [C_IN, N_J * C_OUT], BF16, kind="ExternalInput")
    biasv = nc.dram_tensor("biasv", [128, 4], F32, kind="ExternalInput")
    yt = nc.dram_tensor("yt", [C_OUT, NB], BF16, kind="ExternalOutput")

    Tanh = mybir.ActivationFunctionType.Tanh
    Square = mybir.ActivationFunctionType.Square
    Identity = mybir.ActivationFunctionType.Identity
    ALU_MULT = mybir.AluOpType.mult
    ALU_ADD = mybir.AluOpType.add

    with tile.TileContext(nc) as tc:
        with (
            tc.tile_pool(name="const", bufs=1) as const_pool,
            tc.tile_pool(name="wts", bufs=1) as wpool,
            tc.tile_pool(name="pows", bufs=2) as ppool,
            tc.tile_pool(name="outs", bufs=2) as opool,
            tc.tile_pool(name="psum", bufs=2, space="PSUM") as pspool,
        ):
            # PE warm-up fodder: dummy matmuls on a memset tile hold the
            # HAM clock gate at full speed until the real stream is ready.
            dummy = const_pool.tile([128, B_TILE], BF16, tag="dummy")
            nc.gpsimd.memset(dummy[:], 0.0)
            dps = pspool.tile([128, B_TILE], F32, tag="ps3", name="dps")
            for _ in range(12):
                nc.tensor.matmul(
                    dps[:], lhsT=dummy[:, 0:128], rhs=dummy[:],
                    start=True, stop=True,
                )

            # ALL DMA rides the single Sync HWDGE ring, posted in need-time
            # order. One ring (a) stops the 16 shared DMA engines from
            # round-robining between queues right when the critical
            # sliver-0/W stream must land, (b) keeps DMA doorbell posts off
            # the Scalar queue (they head-of-line blocked the first tanh
            # ~2.5us behind a post that waited on a completion), and (c)
            # drops 32 per-ring-engine semaphores from the NEFF epilogue,
            # which resets each one individually at ~115ns on the
            # HAM-throttled post-stream clock.
            w_sb = {}

            def w_src(ib):
                return wmat.ap()[ib * 128 : (ib + 1) * 128, :].rearrange(
                    "p (j o) -> p j o", j=N_J
                )

            def load_w(ib, m, eng):
                wc = wpool.tile(
                    [128, 2, C_OUT], BF16, tag=f"w{ib}_{m}", name=f"w{ib}_{m}"
                )
                eng.dma_start(out=wc[:], in_=w_src(ib)[:, 2 * m : 2 * m + 2, :])
                w_sb[ib, 2 * m] = (wc, 0)
                w_sb[ib, 2 * m + 1] = (wc, 1)

            xlbs = []
            xlb0 = ppool.tile([128, N_IB, B_TILE], F32, tag="xlb0", bufs=1)

            def load_sliver(ib, eng):
                eng.dma_start(out=xlb0[:, ib, :], in_=xh.ap()[:, 0, ib, :])

            bias_t = const_pool.tile([128, 4], F32)
            load_sliver(0, nc.sync)
            load_w(0, 0, nc.sync)
            load_w(0, 1, nc.sync)
            # bias needed only at the first evacuation (~28us): post it
            # behind the critical sliver-0/first-W prefix
            nc.sync.dma_start(out=bias_t[:], in_=biasv.ap())
            load_sliver(1, nc.sync)
            load_w(0, 2, nc.sync)
            load_w(0, 3, nc.sync)
            load_sliver(2, nc.sync)
            load_w(1, 0, nc.sync)
            load_w(1, 1, nc.sync)
            load_sliver(3, nc.sync)
            load_w(1, 2, nc.sync)
            load_w(1, 3, nc.sync)
            for ib in range(2, N_IB):
                for m in range(N_J // 2):
                    load_w(ib, m, nc.sync)
            xlbs.append(xlb0)

            # x phases 1-3 ride BEHIND all W: not needed until ~39/66/93us.
            for ph in range(1, N_PHASES):
                xlb = ppool.tile(
                    [128, N_IB, B_TILE], F32, tag=f"xlb{ph}", bufs=1,
                    name=f"xlb{ph}",
                )
                nc.sync.dma_start(out=xlb[:], in_=xh.ap()[:, ph])
                xlbs.append(xlb)

            def w_chunk(ib, j, oc):
                wc, slot = w_sb[ib, j]
                return wc[:, slot, oc * 128 : (oc + 1) * 128]

            cm1 = const_pool.tile([128, 1], F32, tag="cm1")
            nc.vector.memset(cm1[:], -1.0)

            # ordering guard: each block's first tanh takes a zero-column
            # bias derived from the previous block's LAST ACT output (f6),
            # so the scheduler cannot hoist a DMA-gated tanh ahead of the
            # running chain and head-of-line block the in-order ACT queue
            # on a late transfer
            guard_prev = None

            for ph in range(N_PHASES):
                ps = [
                    pspool.tile([128, B_TILE], F32, tag=f"ps{oc}", name=f"ps{oc}_{ph}")
                    for oc in range(4)
                ]
                bsl = slice(ph * B_TILE, (ph + 1) * B_TILE)
                xlb = xlbs[ph]
                for ib in range(N_IB):
                    # fp32 basis chain; ACT ops in program order:
                    # tanh, tanh, f2, f4, f8, f6 (f6 last: it needs DVE t3)
                    if guard_prev is None:
                        nc.scalar.activation(xlb[:, ib, :], xlb[:, ib, :], Tanh)
                    else:
                        zc = ppool.tile([128, 1], F32, tag="zc", bufs=3)
                        nc.gpsimd.tensor_scalar(
                            zc[:], guard_prev[:, 0:1], 0.0, 0.0, ALU_MULT, ALU_ADD
                        )
                        nc.scalar.activation(
                            xlb[:, ib, :], xlb[:, ib, :], Tanh, bias=zc[:]
                        )
                    t1 = ppool.tile([128, B_TILE], F32, tag="t1", bufs=3)
                    nc.scalar.activation(t1[:], xlb[:, ib, :], Tanh)
                    f2 = ppool.tile([128, B_TILE], F32, tag="f2", bufs=3)
                    nc.scalar.activation(f2[:], t1[:], Square)
                    f4 = ppool.tile([128, B_TILE], F32, tag="f4", bufs=3)
                    nc.scalar.activation(f4[:], f2[:], Square, bias=cm1[:], scale=2.0)
                    # f4b on ACT (it has queue slack; DVE is the fuller
                    # engine) and right behind f4, so the jj=2 matmuls never
                    # wait on the DVE product chain
                    f4b = ppool.tile([128, B_TILE], BF16, tag="f4b", bufs=3)
                    nc.scalar.activation(f4b[:], f4[:], Identity)
                    f8 = ppool.tile([128, B_TILE], BF16, tag="f8", bufs=3)
                    nc.scalar.activation(f8[:], f4[:], Square, bias=cm1[:], scale=2.0)

                    # DVE, in matmul consumption order: the two feed casts
                    # first, then affines and products as their deps land
                    t1b = ppool.tile([128, B_TILE], BF16, tag="t1b", bufs=3)
                    nc.vector.tensor_copy(t1b[:], t1[:])
                    f2b = ppool.tile([128, B_TILE], BF16, tag="f2b", bufs=3)
                    nc.vector.tensor_copy(f2b[:], f2[:])
                    t2 = ppool.tile([128, B_TILE], F32, tag="t2", bufs=3)
                    nc.vector.tensor_scalar(t2[:], f2[:], 2.0, -1.0, ALU_MULT, ALU_ADD)
                    u3 = ppool.tile([128, B_TILE], F32, tag="u3", bufs=3)
                    nc.vector.tensor_scalar(u3[:], f2[:], 4.0, -3.0, ALU_MULT, ALU_ADD)
                    f3 = ppool.tile([128, B_TILE], BF16, tag="f3", bufs=3)
                    nc.vector.tensor_mul(f3[:], t1[:], t2[:])
                    t3 = ppool.tile([128, B_TILE], F32, tag="t3", bufs=3)
                    nc.vector.tensor_mul(t3[:], t1[:], u3[:])
                    f5 = ppool.tile([128, B_TILE], BF16, tag="f5", bufs=3)
                    nc.vector.tensor_mul(f5[:], t2[:], t3[:])
                    t4 = ppool.tile([128, B_TILE], F32, tag="t4", bufs=3)
                    nc.vector.tensor_scalar(t4[:], f4[:], 2.0, -1.0, ALU_MULT, ALU_ADD)
                    f6 = ppool.tile([128, B_TILE], BF16, tag="f6", bufs=3)
                    nc.scalar.activation(f6[:], t3[:], Square)
                    guard_prev = f6
                    f7 = ppool.tile([128, B_TILE], BF16, tag="f7", bufs=3)
                    nc.vector.tensor_mul(f7[:], t3[:], t4[:])

                    basis = [t1b, f2b, f3, f4b, f5, f6, f7, f8]
                    # device column jj consumes basis function J_ORDER[jj]
                    chunks = [basis[J_ORDER[jj]] for jj in range(N_J)]
                    out_eng = nc.sync

                    def evac(oc, csl):
                        osb = opool.tile(
                            [128, B_TILE], BF16, tag=f"osb{oc}", name=f"osb{oc}"
                        )
                        nc.scalar.activation(
                            osb[:, csl], ps[oc][:, csl], Identity,
                            bias=bias_t[:, oc : oc + 1],
                        )
                        out_eng.dma_start(
                            out=yt.ap()[
                                oc * 128 : (oc + 1) * 128,
                                ph * B_TILE + (csl.start or 0) : ph * B_TILE
                                + (csl.stop or B_TILE),
                            ],
                            in_=osb[:, csl],
                        )
                        return osb

                    if ib < N_IB - 1:
                        for jj in range(N_J):
                            for oc in range(4):
                                nc.tensor.matmul(
                                    ps[oc][:],
                                    lhsT=w_chunk(ib, jj, oc),
                                    rhs=chunks[jj][:],
                                    start=(ib == 0 and jj == 0),
                                    stop=False,
                                )
                    else:
                        # oc-major on the last row-block: accumulation groups
                        # finish staggered -> evacuation overlaps matmuls.
                        for oc in range(3):
                            for jj in range(N_J):
                                nc.tensor.matmul(
                                    ps[oc][:],
                                    lhsT=w_chunk(ib, jj, oc),
                                    rhs=chunks[jj][:],
                                    start=False,
                                    stop=(jj == N_J - 1),
                                )
                            evac(oc, slice(0, B_TILE))
                        # final group in column halves so the very last
                        # evacuation + store move only 64KB; evacuations run
                        # after both halves so the PE never waits on an ACT
                        # read of the still-accumulating PSUM bank
                        for half in range(2):
                            csl = slice(half * 256, half * 256 + 256)
                            for jj in range(N_J):
                                nc.tensor.matmul(
                                    ps[3][:, csl],
                                    lhsT=w_chunk(ib, jj, 3),
                                    rhs=chunks[jj][:, csl],
                                    start=False,
                                    stop=(jj == N_J - 1 and half == 1),
                                    skip_group_check=True,
                                )
                        osb_half_a = evac(3, slice(0, 256))
                        evac(3, slice(256, 512))
                        if ph == N_PHASES - 1:
                            # hold the clock gate through the tail: a few
                            # dummy matmuls gated on the final evacuation
                            # (fresh ps0-tag tile = phase-2's long-idle bank)
                            dps2 = pspool.tile(
                                [128, B_TILE], F32, tag="ps0", name="dps2"
                            )
                            for _ in range(5):
                                nc.tensor.matmul(
                                    dps2[:, 0:256], lhsT=dummy[:, 0:128],
                                    rhs=osb_half_a[:, 0:256],
                                    start=True, stop=True,
                                )
    nc.compile()
    return nc


def _host_transform(cheby_coeffs):
    # Map Chebyshev coefficients onto the device phi basis:
    # phi = [T1, T1^2, T1*T2, T2^2, T2*T3, T3^2, T3*T4, T4^2] and a constant.
    # T_{2k} = 2*T_k^2 - 1, T_{m+n} = 2*T_m*T_n - T_{m-n} =>
    #   y = bias + (C1-C3-C5-C7)*T1 + sum_{d=2..8} 2*C_d * phi_{d-1}
    #   bias_o = sum_i (C0 - C2 - C4 - C6 - C8)
    C64 = cheby_coeffs.astype(np.float64)
    bias = (C64[..., 0] - C64[..., 2] - C64[..., 4] - C64[..., 6] - C64[..., 8]).sum(
        axis=0
    )
    W = np.empty((C_IN, C_OUT, N_J), np.float64)
    W[..., 0] = C64[..., 1] - C64[..., 3] - C64[..., 5] - C64[..., 7]
    for d in range(2, DEGREE + 1):
        W[..., d - 1] = 2.0 * C64[..., d]
    # [i, jj*512+o] with the basis axis permuted into device consumption
    # order (J_ORDER); per-partition-contiguous coefficient rows, bf16
    Wp = W[:, :, J_ORDER]
    Wd = np.ascontiguousarray(
        Wp.transpose(0, 2, 1).reshape(C_IN, N_J * C_OUT).astype(ml_dtypes.bfloat16)
    )
    bias_dev = np.ascontiguousarray(bias.reshape(4, 128).T.astype(np.float32))
    return Wd, bias_dev


def _dev_inputs(x, cheby_coeffs):
    Wd, bias_dev = _host_transform(cheby_coeffs)
    in_maps = []
    for c in range(N_CORES):
        xc = x[c * NB : (c + 1) * NB, :]  # [2048, 512]
        # [p, ph, ib, b] with p the SBUF partition (channel i = ib*128+p)
        xhc = np.ascontiguousarray(
            xc.reshape(N_PHASES, B_TILE, N_IB, 128).transpose(3, 0, 2, 1)
        )
        in_maps.append({"xh": xhc, "wmat": Wd, "biasv": bias_dev})
    return in_maps


def kernel(x, cheby_coeffs):
    x = np.asarray(x, dtype=np.float32)
    cheby_coeffs = np.asarray(cheby_coeffs, dtype=np.float32)
    if "nc" not in _CACHE:
        _CACHE["nc"] = _build()
    nc = _CACHE["nc"]

    in_maps = _dev_inputs(x, cheby_coeffs)
    res = run_bass_kernel_spmd(nc, in_maps, core_ids=list(range(N_CORES)))
    y = np.concatenate(
        [res.results[c]["yt"].T.astype(np.float32) for c in range(N_CORES)], axis=0
    )
    return y



# revision 12
# speedup vs baseline: 1.0299x; 1.0033x over previous
"""ChebyKAN linear layer on 8 Trainium2 NeuronCores.

Math: y[b,o] = sum_{i,d} T_d(w[b,i]) * C[i,o,d], with w = tanh(tanh(x)) and
T_d the Chebyshev polynomials. The device evaluates the Chebyshev-product
basis phi = [T1, T1^2, T1*T2, T2^2, T2*T3, T3^2, T3*T4, T4^2]; an exact
host-side linear transform maps Chebyshev coefficients onto this basis
(constant column folds into a per-o bias added during PSUM evacuation),
with the basis axis permuted into the device consumption order J_ORDER.

Sharding: data-parallel over batch b (16384 -> 2048/core); coeffs
replicated. x is pre-laid-out on the host as [128 part, phase, ib, 512].

Matmul operands are bf16 (1 cycle/row like f32r at free-dim 512, but half
the DMA + SBUF read traffic and 2x faster FWL weight loads). The basis
chain stays fp32 on ACT/DVE; each matmul operand is rounded to bf16
exactly once. Output is stored bf16 and widened on the host. Measured
rel err 4.5e-3 (gate 2e-2); PE stream runs at 216 ns per 512-row matmul
vs the 213.3 ns roofline.

Engine layout per (phase, ib) block:
  ACT: tanh, tanh, Sq(t1)=f2, Sq(2f2-1)=f4, Sq(2f4-1)=f8, Sq(t3)=f6
       (the two affines fold into the Square pre-scale/bias)
  DVE: casts t1b/f2b/f4b, TS t2/u3/t4, TT t3/f3/f5/f7, and a 1-column
       zero "guard" per block (see below)
  PE:  8 basis cols x 4 output blocks, j-major; last row-block oc-major
       so the four PSUM groups finish staggered and evacuation overlaps
       the stream; the final group runs in column halves so the very
       last store moves only 64KB

Hard-won scheduling rules encoded here (the Tile scheduler's DMA model is
optimistic, engine queues are in-order, and HAM re-throttles the core ~2us
after any PE idle):
  - every DMA-gated ACT op (the in-place tanh of each block) takes a bias
    operand derived from the previous block's last ACT output, so the
    scheduler cannot hoist it ahead of the running chain and head-of-line
    block the ACT queue on a late transfer
  - x phases 1-3 ride the Sync ring BEHIND all W: anything sizable on the
    Pool SWDGE ring early starves the shared DMA-engine pool exactly when
    the critical W stream needs it
  - warm-up dummy matmuls cover the PE from engine-ready (~7.4us) to the
    first real matmul (~13us) so the clock gate never drops; a few more
    dummies gated on the first half-evacuation hold full clock through the
    tail without extending the PE drain past the final store

Residual (measured, near-irreducible here): ~8.3us runtime preamble before
the first DMA packet, ~3.3us first-sliver transfer + completion semaphore,
~1.6us serial tanh/square chain ramp (the post-gap matmuls all wait on the
DVE basis semaphore, not weights), 2.7ns/matmul over the PE roofline, and
~3.4us TileContext teardown.
"""

import sys

if "/opt/trn_rl_repo" not in sys.path:
    sys.path.append("/opt/trn_rl_repo")

import ml_dtypes
import numpy as np

import concourse.bacc as bacc
import concourse.tile as tile
from concourse import mybir
from concourse.bass_utils import run_bass_kernel_spmd

DEGREE = 8
B, C_IN, C_OUT = 16384, 512, 512
N_CORES = 8
NB = B // N_CORES            # 2048 batch rows per core
B_TILE = 512                 # batch window per PSUM accumulation phase
N_PHASES = NB // B_TILE      # 4
N_IB = C_IN // 128           # 4 contraction row-blocks
N_J = DEGREE                 # basis funcs phi_1..phi_8 (constant -> bias)
F32 = mybir.dt.float32
F16 = mybir.dt.float16
BF16 = mybir.dt.bfloat16

_CACHE = {}

# per-ib matmul consumption order of the basis functions, by readiness:
# t1b, f2b first, then the fused-ACT f4/f8 and the DVE products
J_ORDER = [0, 1, 3, 2, 7, 4, 5, 6]


def _build():
    nc = bacc.Bacc("TRN2", target_bir_lowering=False, debug=False)
    xh = nc.dram_tensor("xh", [128, N_PHASES, N_IB, B_TILE], BF16, kind="ExternalInput")
    wmat = nc.dram_tensor("wmat", [C_IN, N_J * C_OUT], BF16, kind="ExternalInput")
    biasv = nc.dram_tensor("biasv", [128, 4], F32, kind="ExternalInput")
    yt = nc.dram_tensor("yt", [C_OUT, NB], BF16, kind="ExternalOutput")

    Tanh = mybir.ActivationFunctionType.Tanh
    Square = mybir.ActivationFunctionType.Square
    Identity = mybir.ActivationFunctionType.Identity
    ALU_MULT = mybir.AluOpType.mult
    ALU_ADD = mybir.AluOpType.add

    with tile.TileContext(nc) as tc:
        with (
            tc.tile_pool(name="const", bufs=1) as const_pool,
            tc.tile_pool(name="wts", bufs=1) as wpool,
            tc.tile_pool(name="pows", bufs=2) as ppool,
            tc.tile_pool(name="outs", bufs=2) as opool,
            tc.tile_pool(name="psum", bufs=2, space="PSUM") as pspool,
        ):
            # PE warm-up fodder: dummy matmuls on a memset tile hold the
            # HAM clock gate at full speed until the real stream is ready.
            dummy = const_pool.tile([128, B_TILE], BF16, tag="dummy")
            nc.gpsimd.memset(dummy[:], 0.0)
            dps = pspool.tile([128, B_TILE], F32, tag="ps3", name="dps")
            for _ in range(9):
                nc.tensor.matmul(
                    dps[:], lhsT=dummy[:, 0:128], rhs=dummy[:],
                    start=True, stop=True,
                )

            # ALL DMA rides the single Sync HWDGE ring, posted in need-time
            # order. One ring (a) stops the 16 shared DMA engines from
            # round-robining between queues right when the critical
            # sliver-0/W stream must land, (b) keeps DMA doorbell posts off
            # the Scalar queue (they head-of-line blocked the first tanh
            # ~2.5us behind a post that waited on a completion), and (c)
            # drops 32 per-ring-engine semaphores from the NEFF epilogue,
            # which resets each one individually at ~115ns on the
            # HAM-throttled post-stream clock.
            w_sb = {}

            def w_src(ib):
                return wmat.ap()[ib * 128 : (ib + 1) * 128, :].rearrange(
                    "p (j o) -> p j o", j=N_J
                )

            def load_w(ib, m, eng):
                wc = wpool.tile(
                    [128, 2, C_OUT], BF16, tag=f"w{ib}_{m}", name=f"w{ib}_{m}"
                )
                eng.dma_start(out=wc[:], in_=w_src(ib)[:, 2 * m : 2 * m + 2, :])
                w_sb[ib, 2 * m] = (wc, 0)
                w_sb[ib, 2 * m + 1] = (wc, 1)

            xlbs = []
            xlb0 = ppool.tile([128, N_IB, B_TILE], BF16, tag="xlb0", bufs=1)

            def load_sliver(ib, eng):
                eng.dma_start(out=xlb0[:, ib, :], in_=xh.ap()[:, 0, ib, :])

            bias_t = const_pool.tile([128, 4], F32)
            load_sliver(0, nc.sync)
            load_w(0, 0, nc.sync)
            load_w(0, 1, nc.sync)
            # bias needed only at the first evacuation (~28us): post it
            # behind the critical sliver-0/first-W prefix
            nc.sync.dma_start(out=bias_t[:], in_=biasv.ap())
            load_sliver(1, nc.sync)
            load_w(0, 2, nc.sync)
            load_w(0, 3, nc.sync)
            load_sliver(2, nc.sync)
            load_w(1, 0, nc.sync)
            load_w(1, 1, nc.sync)
            load_sliver(3, nc.sync)
            load_w(1, 2, nc.sync)
            load_w(1, 3, nc.sync)
            for ib in range(2, N_IB):
                for m in range(N_J // 2):
                    load_w(ib, m, nc.sync)
            xlbs.append(xlb0)

            # x phases 1-3 ride BEHIND all W: not needed until ~39/66/93us.
            for ph in range(1, N_PHASES):
                xlb = ppool.tile(
                    [128, N_IB, B_TILE], BF16, tag=f"xlb{ph}", bufs=1,
                    name=f"xlb{ph}",
                )
                nc.sync.dma_start(out=xlb[:], in_=xh.ap()[:, ph])
                xlbs.append(xlb)

            def w_chunk(ib, j, oc):
                wc, slot = w_sb[ib, j]
                return wc[:, slot, oc * 128 : (oc + 1) * 128]

            cm1 = const_pool.tile([128, 1], F32, tag="cm1")
            nc.vector.memset(cm1[:], -1.0)

            # ordering guard: each block's first tanh takes a zero-column
            # bias derived from the previous block's LAST ACT output (f6),
            # so the scheduler cannot hoist a DMA-gated tanh ahead of the
            # running chain and head-of-line block the in-order ACT queue
            # on a late transfer
            guard_prev = None

            for ph in range(N_PHASES):
                ps = [
                    pspool.tile([128, B_TILE], F32, tag=f"ps{oc}", name=f"ps{oc}_{ph}")
                    for oc in range(4)
                ]
                bsl = slice(ph * B_TILE, (ph + 1) * B_TILE)
                xlb = xlbs[ph]
                for ib in range(N_IB):
                    # fp32 basis chain; ACT ops in program order:
                    # tanh, tanh, f2, f4, f8, f6 (f6 last: it needs DVE t3)
                    if guard_prev is None:
                        nc.scalar.activation(xlb[:, ib, :], xlb[:, ib, :], Tanh)
                    else:
                        zc = ppool.tile([128, 1], F32, tag="zc", bufs=3)
                        nc.gpsimd.tensor_scalar(
                            zc[:], guard_prev[:, 0:1], 0.0, 0.0, ALU_MULT, ALU_ADD
                        )
                        nc.scalar.activation(
                            xlb[:, ib, :], xlb[:, ib, :], Tanh, bias=zc[:]
                        )
                    t1 = ppool.tile([128, B_TILE], F32, tag="t1", bufs=3)
                    nc.scalar.activation(t1[:], xlb[:, ib, :], Tanh)
                    f2 = ppool.tile([128, B_TILE], F32, tag="f2", bufs=3)
                    nc.scalar.activation(f2[:], t1[:], Square)
                    f4 = ppool.tile([128, B_TILE], F32, tag="f4", bufs=3)
                    nc.scalar.activation(f4[:], f2[:], Square, bias=cm1[:], scale=2.0)
                    # f4b on ACT (it has queue slack; DVE is the fuller
                    # engine) and right behind f4, so the jj=2 matmuls never
                    # wait on the DVE product chain
                    f4b = ppool.tile([128, B_TILE], BF16, tag="f4b", bufs=3)
                    nc.scalar.activation(f4b[:], f4[:], Identity)
                    f8 = ppool.tile([128, B_TILE], BF16, tag="f8", bufs=3)
                    nc.scalar.activation(f8[:], f4[:], Square, bias=cm1[:], scale=2.0)

                    # DVE, in matmul consumption order: the two feed casts
                    # first, then affines and products as their deps land
                    t1b = ppool.tile([128, B_TILE], BF16, tag="t1b", bufs=3)
                    nc.vector.tensor_copy(t1b[:], t1[:])
                    f2b = ppool.tile([128, B_TILE], BF16, tag="f2b", bufs=3)
                    nc.vector.tensor_copy(f2b[:], f2[:])
                    t2 = ppool.tile([128, B_TILE], F32, tag="t2", bufs=3)
                    nc.vector.tensor_scalar(t2[:], f2[:], 2.0, -1.0, ALU_MULT, ALU_ADD)
                    u3 = ppool.tile([128, B_TILE], F32, tag="u3", bufs=3)
                    nc.vector.tensor_scalar(u3[:], f2[:], 4.0, -3.0, ALU_MULT, ALU_ADD)
                    f3 = ppool.tile([128, B_TILE], BF16, tag="f3", bufs=3)
                    nc.vector.tensor_mul(f3[:], t1[:], t2[:])
                    t3 = ppool.tile([128, B_TILE], F32, tag="t3", bufs=3)
                    nc.vector.tensor_mul(t3[:], t1[:], u3[:])
                    f5 = ppool.tile([128, B_TILE], BF16, tag="f5", bufs=3)
                    nc.vector.tensor_mul(f5[:], t2[:], t3[:])
                    t4 = ppool.tile([128, B_TILE], F32, tag="t4", bufs=3)
                    nc.vector.tensor_scalar(t4[:], f4[:], 2.0, -1.0, ALU_MULT, ALU_ADD)
                    f6 = ppool.tile([128, B_TILE], BF16, tag="f6", bufs=3)
                    nc.scalar.activation(f6[:], t3[:], Square)
                    guard_prev = f6
                    f7 = ppool.tile([128, B_TILE], BF16, tag="f7", bufs=3)
                    nc.vector.tensor_mul(f7[:], t3[:], t4[:])

                    basis = [t1b, f2b, f3, f4b, f5, f6, f7, f8]
                    # device column jj consumes basis function J_ORDER[jj]
                    chunks = [basis[J_ORDER[jj]] for jj in range(N_J)]
                    out_eng = nc.sync

                    def evac(oc, csl, eng=None):
                        osb = opool.tile(
                            [128, B_TILE], BF16, tag=f"osb{oc}", name=f"osb{oc}"
                        )
                        nc.scalar.activation(
                            osb[:, csl], ps[oc][:, csl], Identity,
                            bias=bias_t[:, oc : oc + 1],
                        )
                        (eng or out_eng).dma_start(
                            out=yt.ap()[
                                oc * 128 : (oc + 1) * 128,
                                ph * B_TILE + (csl.start or 0) : ph * B_TILE
                                + (csl.stop or B_TILE),
                            ],
                            in_=osb[:, csl],
                        )
                        return osb

                    if ib < N_IB - 1:
                        for jj in range(N_J):
                            for oc in range(4):
                                nc.tensor.matmul(
                                    ps[oc][:],
                                    lhsT=w_chunk(ib, jj, oc),
                                    rhs=chunks[jj][:],
                                    start=(ib == 0 and jj == 0),
                                    stop=False,
                                )
                    else:
                        # oc-major on the last row-block: accumulation groups
                        # finish staggered -> evacuation overlaps matmuls.
                        last_ph = ph == N_PHASES - 1
                        for oc in range(3):
                            for jj in range(N_J):
                                nc.tensor.matmul(
                                    ps[oc][:],
                                    lhsT=w_chunk(ib, jj, oc),
                                    rhs=chunks[jj][:],
                                    start=False,
                                    stop=(jj == N_J - 1),
                                )
                            # last phase: earlier stores ride the idle Pool
                            # SWDGE ring so the very last store starts on an
                            # empty Sync FIFO and completes ~1us sooner
                            evac(oc, slice(0, B_TILE),
                                 nc.gpsimd if last_ph else None)
                        # final group in column halves so the very last
                        # evacuation + store move only 64KB; evacuations run
                        # after both halves so the PE never waits on an ACT
                        # read of the still-accumulating PSUM bank
                        for half in range(2):
                            csl = slice(half * 256, half * 256 + 256)
                            for jj in range(N_J):
                                nc.tensor.matmul(
                                    ps[3][:, csl],
                                    lhsT=w_chunk(ib, jj, 3),
                                    rhs=chunks[jj][:, csl],
                                    start=False,
                                    stop=(jj == N_J - 1 and half == 1),
                                    skip_group_check=True,
                                )
                        osb_half_a = evac(3, slice(0, 256),
                                          nc.gpsimd if last_ph else None)
                        evac(3, slice(256, 512))
                        if last_ph:
                            # hold the clock gate through the tail: dummy
                            # matmuls gated on the first half-evacuation
                            # (fresh ps0-tag tile = phase-2's long-idle
                            # bank) run until roughly when the final store
                            # completes, so the barrier + NEFF semaphore
                            # epilogue start at full clock
                            dps2 = pspool.tile(
                                [128, B_TILE], F32, tag="ps0", name="dps2"
                            )
                            for _ in range(8):
                                nc.tensor.matmul(
                                    dps2[:, 0:256], lhsT=dummy[:, 0:128],
                                    rhs=osb_half_a[:, 0:256],
                                    start=True, stop=True,
                                )
    nc.compile()
    return nc


def _host_transform(cheby_coeffs):
    # Map Chebyshev coefficients onto the device phi basis:
    # phi = [T1, T1^2, T1*T2, T2^2, T2*T3, T3^2, T3*T4, T4^2] and a constant.
    # T_{2k} = 2*T_k^2 - 1, T_{m+n} = 2*T_m*T_n - T_{m-n} =>
    #   y = bias + (C1-C3-C5-C7)*T1 + sum_{d=2..8} 2*C_d * phi_{d-1}
    #   bias_o = sum_i (C0 - C2 - C4 - C6 - C8)
    C64 = cheby_coeffs.astype(np.float64)
    bias = (C64[..., 0] - C64[..., 2] - C64[..., 4] - C64[..., 6] - C64[..., 8]).sum(
        axis=0
    )
    W = np.empty((C_IN, C_OUT, N_J), np.float64)
    W[..., 0] = C64[..., 1] - C64[..., 3] - C64[..., 5] - C64[..., 7]
    for d in range(2, DEGREE + 1):
        W[..., d - 1] = 2.0 * C64[..., d]
    # [i, jj*512+o] with the basis axis permuted into device consumption
    # order (J_ORDER); per-partition-contiguous coefficient rows, bf16
    Wp = W[:, :, J_ORDER]
    Wd = np.ascontiguousarray(
        Wp.transpose(0, 2, 1).reshape(C_IN, N_J * C_OUT).astype(ml_dtypes.bfloat16)
    )
    bias_dev = np.ascontiguousarray(bias.reshape(4, 128).T.astype(np.float32))
    return Wd, bias_dev


def _dev_inputs(x, cheby_coeffs):
    Wd, bias_dev = _host_transform(cheby_coeffs)
    in_maps = []
    for c in range(N_CORES):
        xc = x[c * NB : (c + 1) * NB, :]  # [2048, 512]
        # [p, ph, ib, b] with p the SBUF partition (channel i = ib*128+p);
        # bf16 halves the critical first-sliver transfer (rel-err impact
        # simulated at 5.5e-3 vs the 2e-2 gate)
        xhc = np.ascontiguousarray(
            xc.reshape(N_PHASES, B_TILE, N_IB, 128)
            .transpose(3, 0, 2, 1)
            .astype(ml_dtypes.bfloat16)
        )
        in_maps.append({"xh": xhc, "wmat": Wd, "biasv": bias_dev})
    return in_maps


def kernel(x, cheby_coeffs):
    x = np.asarray(x, dtype=np.float32)
    cheby_coeffs = np.asarray(cheby_coeffs, dtype=np.float32)
    if "nc" not in _CACHE:
        _CACHE["nc"] = _build()
    nc = _CACHE["nc"]

    in_maps = _dev_inputs(x, cheby_coeffs)
    res = run_bass_kernel_spmd(nc, in_maps, core_ids=list(range(N_CORES)))
    y = np.concatenate(
        [res.results[c]["yt"].T.astype(np.float32) for c in range(N_CORES)], axis=0
    )
    return y

